# revision 23
# baseline (speedup 1.0000x reference)
"""Trainium2 Bass kernel for ContinuousWaveletLayer (CWT energy).

Reference computation:
  bank = Morlet wavelet bank [32 scales, Lmax=256] (static)
  coef[b,s,t] = 'same' conv of x[b,:] (len 8192) with bank[s,:]
  out[b,s]    = mean_t(coef^2) * softmax(scale_weights)[s]

Device strategy (8 NeuronCores, scale-parallel, 4 scales/core):
  The conv is phrased as Toeplitz matmuls on the tensor engine. With x
  zero-padded (128 left / 128 right) and viewed time-major in blocks of
  128, the output block B (128 time steps) for scale s is

      coef_B[to, b] = sum_{D=0..2} G[s,D].T @ Xblk[B+D]          (K=128)

  where G[s,D][a,to] = g_s[128*D + a - to] (g_s = reversed bank row,
  zero outside [0,256)) and Xblk[A][a,b] = xpad[128*A + a, b].
  x is stored in SBUF as [a=128 partitions, (A,b) free], so the rhs for
  (D, 4-block group) is just a contiguous 512-wide free-dim slice; the
  3 D-matmuls accumulate in PSUM.  Squares are computed on the scalar /
  vector engines (alternating) and accumulated in SBUF; the final
  partition reduction is a ones-vector matmul.  Host applies the final
  1/N and softmax scaling on the gathered [32,128] sums (O(4K) flops).
"""

import os
import sys
from contextlib import ExitStack

import numpy as np

sys.path.insert(0, "/opt/trn_rl_repo")

import concourse.bass as bass
import concourse.mybir as mybir
from concourse import tile
from concourse.bass_utils import run_bass_kernel_spmd
from concourse.vector_clock import ScopedClock


def _drain_and_barrier_single_wait(self, tick_clock, wait_clock):
    """TileContext._drain_and_barrier, but the kernel-tail drain's
    global-clock waits are spread over a chain of single-wait drains —
    the walrus build here allows only one sync wait per instruction."""
    drain_inst = self.nc.sync.drain()
    wait_clock.add_sem_waits(
        drain_inst.ins, ScopedClock({None: tick_clock.global_clock})
    )
    si = drain_inst.ins.sync_info
    waits = list(si.on_wait)
    if len(waits) > 1:
        si.on_wait = [waits[0]]
        sems = {h.name: h for h in self.sems.allocated().values()}
        for w in waits[1:]:
            d2 = self.nc.sync.drain()
            d2.wait_op(sems[w.ant_name], w.wait_value, "sem-ge")
    self.nc.all_engine_barrier()
    assert self.sems is not None
    popped = self.nc._tile_sem_poison_stack.pop()
    assert popped is self._sem_poison
    self.nc.clear_and_free_semaphores(list(self.sems.allocated().values()))
    self.nc.all_engine_barrier()


tile.TileContext._drain_and_barrier = _drain_and_barrier_single_wait

N_CORES = 8
S_TOTAL = 32          # number of scales
S_PER = 4             # scales per core
P = 128               # partition / block size
NT = 8192             # time samples
LMAX = 256            # padded kernel length
NBLK = 66             # input blocks: (128 + 8192 + 128) / 128
NOUT = 64             # output blocks: 8192 / 128
NGRP = 16             # groups of 4 output blocks (N=512 matmuls)
F32 = mybir.dt.float32

LAST_RESULTS = None   # BassKernelResults of the most recent run (for test.py)


def _morlet_kernel_bank(n_scales: int, n: int) -> np.ndarray:
    Lmax = min(8 * n_scales, n)
    bank = np.zeros((n_scales, Lmax), dtype=np.float32)
    for i, s in enumerate(range(1, n_scales + 1)):
        L = min(8 * s, n)
        t = np.linspace(-4.0 * s, 4.0 * s, L)
        w = np.exp(-t**2 / (2.0 * s**2)) * np.cos(5.0 * t / s)
        w = w / np.sqrt(s)
        off = (Lmax - 1) // 2 - (L - 1) // 2
        bank[i, off : off + L] = w.astype(np.float32)
    return bank


def _toeplitz_weights() -> np.ndarray:
    """G[s, D][a, to] = g_s[128*D + a - to], zero outside support."""
    bank = _morlet_kernel_bank(S_TOTAL, NT)          # [32, 256]
    g = bank[:, ::-1].copy()                         # reversed rows
    a = np.arange(P)[:, None]
    to = np.arange(P)[None, :]
    G = np.zeros((S_TOTAL, 3, P, P), dtype=np.float32)
    for D in range(3):
        d = 128 * D + a - to
        valid = (d >= 0) & (d < LMAX)
        dc = np.clip(d, 0, LMAX - 1)
        for s in range(S_TOTAL):
            G[s, D] = np.where(valid, g[s][dc], 0.0)
    return G


GCOLS = S_PER * 3 * P          # 1536 weight columns
XCOLS = NBLK * P               # 8448 x columns


def _build_nc() -> bass.Bass:
    nc = bass.Bass()
    # combined input, one DMA → one semaphore lane for every matmul dep:
    #   xg[:, :GCOLS]    = per-core Toeplitz weights (G[s,D,a,to])
    #   xg[:, GCOLS:-1]  = x time-major: xpad[128*A + a, b]
    #   xg[:, -1]        = ones column (partition reducer)
    xg = nc.dram_tensor("xg", [P, GCOLS + XCOLS + 1], F32, kind="ExternalInput")
    # per-core partial energies, un-folded: outp[s, (Bsub, b)]; the host
    # sums the 4 column groups (keeps DVE out of the kernel → fewer
    # semaphore procs for the tail drain)
    outp = nc.dram_tensor("outp", [1, S_PER * 512], F32, kind="ExternalOutput")

    with tile.TileContext(nc) as tc, ExitStack() as ctx:
        xpool = ctx.enter_context(tc.tile_pool(name="x", bufs=1))
        # one sq buffer per (ng, s): no slot reuse → no WAR-induced second
        # wait on the ACT squares (walrus allows 1 sync wait/instruction)
        sqpool = ctx.enter_context(tc.tile_pool(name="sq", bufs=NGRP * S_PER))
        rowpool = ctx.enter_context(tc.tile_pool(name="row", bufs=4))
        pspool = ctx.enter_context(tc.tile_pool(name="ps", bufs=4, space="PSUM"))
        psepool = ctx.enter_context(tc.tile_pool(name="pse", bufs=1, space="PSUM"))

        xgsb = xpool.tile([P, GCOLS + XCOLS + 1], F32)
        # one DMA: a single InstDMACopy fans out across all 16 SDMA engines
        nc.sync.dma_start(out=xgsb[:, :], in_=xg[:, :])
        onesb = xgsb[:, GCOLS + XCOLS : GCOLS + XCOLS + 1]

        # per-scale PSUM energy accumulators [1, (Bsub, b)]
        pes = [
            psepool.tile([1, 512], F32, tag=f"pe{s}", name=f"pe{s}")
            for s in range(S_PER)
        ]

        # main conv loop; all cross-engine deps are 1-wait:
        #   conv matmul:  DMA sem (once) / ACT sem (psum-bank recycle)
        #   ACT square:   PE sem
        #   reduce matmul (PE, accumulates into pes[s]): ACT sem
        for ng in range(NGRP):
            for s in range(S_PER):
                pt = pspool.tile([P, 512], F32)
                for D in range(3):
                    gc = (s * 3 + D) * P
                    xc = GCOLS + (ng * 4 + D) * P
                    lhsT = xgsb[:, gc : gc + P]
                    rhs = xgsb[:, xc : xc + 4 * P]
                    nc.tensor.matmul(
                        pt[:, :], lhsT, rhs, start=(D == 0), stop=(D == 2)
                    )
                sq = sqpool.tile([P, 512], F32)
                nc.scalar.square(sq[:, :], pt[:, :])
                nc.tensor.matmul(
                    pes[s][:, :],
                    onesb,
                    sq[:, :],
                    start=(ng == 0),
                    stop=(ng == NGRP - 1),
                )

        # final: evict the [1,512] accumulators side by side on partition 0
        # (engines can only write at partition base 0), single DMA out
        rowout = rowpool.tile([1, S_PER * 512], F32, tag="rowout", name="rowout")
        for s in range(S_PER):
            nc.scalar.copy(rowout[:, s * 512 : (s + 1) * 512], pes[s][:, :])
        nc.sync.dma_start(out=outp[:, :], in_=rowout[:, :])

    return nc


_NC_CACHE = None


def _get_nc() -> bass.Bass:
    global _NC_CACHE
    if _NC_CACHE is None:
        _NC_CACHE = _build_nc()
    return _NC_CACHE


def kernel(x: np.ndarray, scale_weights: np.ndarray, _trace: bool = False) -> np.ndarray:
    global LAST_RESULTS
    x = np.asarray(x, dtype=np.float32)
    scale_weights = np.asarray(scale_weights, dtype=np.float32)
    assert x.shape == (P, NT) and scale_weights.shape == (S_TOTAL,)

    # host prep: zero-pad, transpose to time-major blocked layout
    xpad = np.zeros((NBLK * P, P), dtype=np.float32)
    xpad[P : P + NT, :] = x.T
    # xb2[a, A*128 + b] = xpad[A*128 + a, b]
    xb2 = np.ascontiguousarray(
        xpad.reshape(NBLK, P, P).transpose(1, 0, 2).reshape(P, NBLK * P)
    )

    G = _toeplitz_weights()  # [32, 3, 128, 128]
    # combined per-core input: [weights | x | ones]; core c handles
    # scales [4c, 4c+4)
    ones = np.ones((P, 1), dtype=np.float32)
    xgs = []
    for c in range(N_CORES):
        Gc = G[c * S_PER : (c + 1) * S_PER].reshape(S_PER * 3, P, P)
        gw2 = Gc.transpose(1, 0, 2).reshape(P, GCOLS)
        xgs.append(np.ascontiguousarray(np.concatenate([gw2, xb2, ones], axis=1)))

    nc = _get_nc()
    in_maps = [{"xg": xgs[c]} for c in range(N_CORES)]
    res = run_bass_kernel_spmd(nc, in_maps, list(range(N_CORES)), trace=_trace)
    LAST_RESULTS = res

    # gather + unshard: [8 cores][1, 4 scales * (4 Bsub * 128 b)] -> [128, 32]
    esum = np.concatenate(
        [res.results[c]["outp"].reshape(S_PER, 512) for c in range(N_CORES)],
        axis=0,
    )  # [32, 512]
    esum = esum.reshape(S_TOTAL, 4, P).sum(axis=1)  # fold Bsub -> [32, 128]
    energy = esum.T / np.float32(NT)

    w = scale_weights.astype(np.float64)
    e = np.exp(w - w.max())
    sm = (e / e.sum()).astype(np.float32)
    return (energy * sm[None, :]).astype(np.float32)


if __name__ == "__main__":
    rng = np.random.default_rng(0)
    x = rng.standard_normal((P, NT), dtype=np.float32)
    sw = rng.standard_normal(S_TOTAL, dtype=np.float32)
    out = kernel(x, sw)
    print("kernel output shape:", out.shape, out.dtype)


# revision 28
# speedup vs baseline: 3.1000x; 3.1000x over previous
"""Trainium2 Bass kernel for ContinuousWaveletLayer (CWT energy).

Reference computation:
  bank = Morlet wavelet bank [32 scales, Lmax=256] (static)
  coef[b,s,t] = 'same' conv of x[b,:] (len 8192) with bank[s,:]
  out[b,s]    = mean_t(coef^2) * softmax(scale_weights)[s]

Device strategy (8 NeuronCores, scale-parallel, 4 scales/core):
  The conv is phrased as Toeplitz matmuls on the tensor engine. With x
  zero-padded (128 left / 128 right) and viewed time-major in blocks of
  128, the output block B (128 time steps) for scale s is

      coef_B[to, b] = sum_{D=0..2} G[s,D].T @ Xblk[B+D]          (K=128)

  where G[s,D][a,to] = g_s[128*D + a - to] (g_s = reversed bank row,
  zero outside [0,256)) and Xblk[A][a,b] = xpad[128*A + a, b].
  x is stored in SBUF as [a=128 partitions, (A,b) free], so the rhs for
  (D, 4-block group) is just a contiguous 512-wide free-dim slice; the
  3 D-matmuls accumulate in PSUM.  Squares are computed on the scalar /
  vector engines (alternating) and accumulated in SBUF; the final
  partition reduction is a ones-vector matmul.  Host applies the final
  1/N and softmax scaling on the gathered [32,128] sums (O(4K) flops).
"""

import os
import sys
from contextlib import ExitStack

import numpy as np

sys.path.insert(0, "/opt/trn_rl_repo")

import concourse.bass as bass
import concourse.mybir as mybir
from concourse import tile
from concourse.bass_utils import run_bass_kernel_spmd
from concourse.vector_clock import ScopedClock


def _drain_and_barrier_single_wait(self, tick_clock, wait_clock):
    """TileContext._drain_and_barrier, but the kernel-tail drain's
    global-clock waits are spread over a chain of single-wait drains —
    the walrus build here allows only one sync wait per instruction."""
    drain_inst = self.nc.sync.drain()
    wait_clock.add_sem_waits(
        drain_inst.ins, ScopedClock({None: tick_clock.global_clock})
    )
    si = drain_inst.ins.sync_info
    waits = list(si.on_wait)
    if len(waits) > 1:
        si.on_wait = [waits[0]]
        sems = {h.name: h for h in self.sems.allocated().values()}
        for w in waits[1:]:
            d2 = self.nc.sync.drain()
            d2.wait_op(sems[w.ant_name], w.wait_value, "sem-ge")
    self.nc.all_engine_barrier()
    assert self.sems is not None
    popped = self.nc._tile_sem_poison_stack.pop()
    assert popped is self._sem_poison
    self.nc.clear_and_free_semaphores(list(self.sems.allocated().values()))
    self.nc.all_engine_barrier()


tile.TileContext._drain_and_barrier = _drain_and_barrier_single_wait

N_CORES = 8
S_TOTAL = 32          # number of scales
S_PER = 4             # scales per core
P = 128               # partition / block size
NT = 8192             # time samples
LMAX = 256            # padded kernel length
NBLK = 66             # input blocks: (128 + 8192 + 128) / 128
NOUT = 64             # output blocks: 8192 / 128
NGRP = 16             # groups of 4 output blocks (N=512 matmuls)
F32 = mybir.dt.float32
BF16 = mybir.dt.bfloat16

LAST_RESULTS = None   # BassKernelResults of the most recent run (for test.py)


def _morlet_kernel_bank(n_scales: int, n: int) -> np.ndarray:
    Lmax = min(8 * n_scales, n)
    bank = np.zeros((n_scales, Lmax), dtype=np.float32)
    for i, s in enumerate(range(1, n_scales + 1)):
        L = min(8 * s, n)
        t = np.linspace(-4.0 * s, 4.0 * s, L)
        w = np.exp(-t**2 / (2.0 * s**2)) * np.cos(5.0 * t / s)
        w = w / np.sqrt(s)
        off = (Lmax - 1) // 2 - (L - 1) // 2
        bank[i, off : off + L] = w.astype(np.float32)
    return bank


def _toeplitz_weights() -> np.ndarray:
    """G[s, D][a, to] = g_s[128*D + a - to], zero outside support."""
    bank = _morlet_kernel_bank(S_TOTAL, NT)          # [32, 256]
    g = bank[:, ::-1].copy()                         # reversed rows
    a = np.arange(P)[:, None]
    to = np.arange(P)[None, :]
    G = np.zeros((S_TOTAL, 3, P, P), dtype=np.float32)
    for D in range(3):
        d = 128 * D + a - to
        valid = (d >= 0) & (d < LMAX)
        dc = np.clip(d, 0, LMAX - 1)
        for s in range(S_TOTAL):
            G[s, D] = np.where(valid, g[s][dc], 0.0)
    return G


GCOLS = S_PER * 3 * P          # 1536 weight columns
XCOLS = NBLK * P               # 8448 x columns


def _build_nc() -> bass.Bass:
    nc = bass.Bass()
    # combined input, one DMA → one semaphore lane for every matmul dep:
    #   xg[:, :GCOLS]    = per-core Toeplitz weights (G[s,D,a,to])
    #   xg[:, GCOLS:-1]  = x time-major: xpad[128*A + a, b]
    #   xg[:, -1]        = ones column (partition reducer)
    xg = nc.dram_tensor("xg", [P, GCOLS + XCOLS + 1], BF16, kind="ExternalInput")
    # per-core partial energies, un-folded: outp[s, (Bsub, b)]; the host
    # sums the 4 column groups (keeps DVE out of the kernel → fewer
    # semaphore procs for the tail drain)
    outp = nc.dram_tensor("outp", [1, S_PER * 512], F32, kind="ExternalOutput")

    with tile.TileContext(nc) as tc, ExitStack() as ctx:
        xpool = ctx.enter_context(tc.tile_pool(name="x", bufs=1))
        # one sq buffer per (ng, s): no slot reuse → no WAR-induced second
        # wait on the ACT squares (walrus allows 1 sync wait/instruction)
        sqpool = ctx.enter_context(tc.tile_pool(name="sq", bufs=NGRP * S_PER))
        # fp32 PSUM-evict scratch for the DVE square path; slot WARs are
        # DVE-vs-DVE (same engine) so reuse costs no extra waits
        cppool = ctx.enter_context(tc.tile_pool(name="cp", bufs=4))
        rowpool = ctx.enter_context(tc.tile_pool(name="row", bufs=4))
        pspool = ctx.enter_context(tc.tile_pool(name="ps", bufs=4, space="PSUM"))
        psepool = ctx.enter_context(tc.tile_pool(name="pse", bufs=1, space="PSUM"))

        xgsb = xpool.tile([P, GCOLS + XCOLS + 1], BF16)
        # one DMA: a single InstDMACopy fans out across all 16 SDMA engines
        nc.sync.dma_start(out=xgsb[:, :], in_=xg[:, :])
        onesb = xgsb[:, GCOLS + XCOLS : GCOLS + XCOLS + 1]

        # per-scale PSUM energy accumulators [1, (Bsub, b)]
        pes = [
            psepool.tile([1, 512], F32, tag=f"pe{s}", name=f"pe{s}")
            for s in range(S_PER)
        ]

        # main conv loop; all cross-engine deps are 1-wait:
        #   conv matmul:  DMA sem (once) / evict-engine sem (bank recycle)
        #   evict+square: PE sem (ACT path) or PE sem + DVE-self (DVE path)
        #   reduce matmul (PE, accumulates into pes[s]): ACT/DVE sem
        for ng in range(NGRP):
            for s in range(S_PER):
                pt = pspool.tile([P, 512], F32)
                for D in range(3):
                    gc = (s * 3 + D) * P
                    xc = GCOLS + (ng * 4 + D) * P
                    lhsT = xgsb[:, gc : gc + P]
                    rhs = xgsb[:, xc : xc + 4 * P]
                    nc.tensor.matmul(
                        pt[:, :], lhsT, rhs, start=(D == 0), stop=(D == 2)
                    )
                sq = sqpool.tile([P, 512], BF16)
                if (ng * S_PER + s) % 2 == 0:
                    # ACT path: square+cast straight out of PSUM
                    nc.scalar.square(sq[:, :], pt[:, :])
                else:
                    # DVE path: fp32 copy out of PSUM, then square+cast
                    cp = cppool.tile([P, 512], F32)
                    nc.vector.tensor_copy(cp[:, :], pt[:, :])
                    nc.vector.tensor_mul(sq[:, :], cp[:, :], cp[:, :])
                nc.tensor.matmul(
                    pes[s][:, :],
                    onesb,
                    sq[:, :],
                    start=(ng == 0),
                    stop=(ng == NGRP - 1),
                )

        # final: evict the [1,512] accumulators side by side on partition 0
        # (engines can only write at partition base 0), single DMA out
        rowout = rowpool.tile([1, S_PER * 512], F32, tag="rowout", name="rowout")
        for s in range(S_PER):
            nc.scalar.copy(rowout[:, s * 512 : (s + 1) * 512], pes[s][:, :])
        nc.sync.dma_start(out=outp[:, :], in_=rowout[:, :])

    return nc


_NC_CACHE = None


def _get_nc() -> bass.Bass:
    global _NC_CACHE
    if _NC_CACHE is None:
        _NC_CACHE = _build_nc()
    return _NC_CACHE


def kernel(x: np.ndarray, scale_weights: np.ndarray, _trace: bool = False) -> np.ndarray:
    global LAST_RESULTS
    x = np.asarray(x, dtype=np.float32)
    scale_weights = np.asarray(scale_weights, dtype=np.float32)
    assert x.shape == (P, NT) and scale_weights.shape == (S_TOTAL,)

    # host prep: zero-pad, transpose to time-major blocked layout
    xpad = np.zeros((NBLK * P, P), dtype=np.float32)
    xpad[P : P + NT, :] = x.T
    # xb2[a, A*128 + b] = xpad[A*128 + a, b]
    xb2 = np.ascontiguousarray(
        xpad.reshape(NBLK, P, P).transpose(1, 0, 2).reshape(P, NBLK * P)
    )

    G = _toeplitz_weights()  # [32, 3, 128, 128]
    # combined per-core input: [weights | x | ones], bf16 for the 1-col/cycle
    # matmul stream; core c handles scales [4c, 4c+4)
    import ml_dtypes

    bf16 = ml_dtypes.bfloat16
    ones = np.ones((P, 1), dtype=np.float32)
    xgs = []
    for c in range(N_CORES):
        Gc = G[c * S_PER : (c + 1) * S_PER].reshape(S_PER * 3, P, P)
        gw2 = Gc.transpose(1, 0, 2).reshape(P, GCOLS)
        xgs.append(
            np.ascontiguousarray(
                np.concatenate([gw2, xb2, ones], axis=1).astype(bf16)
            )
        )

    nc = _get_nc()
    in_maps = [{"xg": xgs[c]} for c in range(N_CORES)]
    res = run_bass_kernel_spmd(nc, in_maps, list(range(N_CORES)), trace=_trace)
    LAST_RESULTS = res

    # gather + unshard: [8 cores][1, 4 scales * (4 Bsub * 128 b)] -> [128, 32]
    esum = np.concatenate(
        [res.results[c]["outp"].reshape(S_PER, 512) for c in range(N_CORES)],
        axis=0,
    )  # [32, 512]
    esum = esum.reshape(S_TOTAL, 4, P).sum(axis=1)  # fold Bsub -> [32, 128]
    energy = esum.T / np.float32(NT)

    w = scale_weights.astype(np.float64)
    e = np.exp(w - w.max())
    sm = (e / e.sum()).astype(np.float32)
    return (energy * sm[None, :]).astype(np.float32)


if __name__ == "__main__":
    rng = np.random.default_rng(0)
    x = rng.standard_normal((P, NT), dtype=np.float32)
    sw = rng.standard_normal(S_TOTAL, dtype=np.float32)
    out = kernel(x, sw)
    print("kernel output shape:", out.shape, out.dtype)
